# revision 9
# baseline (speedup 1.0000x reference)
"""Trainium2 Bass kernel for nn_DigitConvolutionalModel (3x3 conv + 3-layer MLP).

Math: out = relu(relu(conv3x3(x) @ W1 + b1) @ W2 + b2) @ W3 + b3.

The 3x3 valid conv is linear, so on host we fold it into the first FC:
  h1 = relu(x @ W1eff + b1)  with  W1eff = A @ W1 : [784, 256].
K = 784 is used EXACTLY (6 full 128-row k-tiles + one 16-row tail tile).
b1 rides the tail matmul as a 17th row (ones row in x-tail, b1 row in
W-tail), so L1 needs no separate bias add.  b2/b3 ride as fp16 columns of
the wa tensor (one DVE copy stages them to fp32).

Sharding: pure data parallel over the batch across 8 cores (2048 rows each).
Feature-major 3-layer MLP (activations transposed; zero on-device transposes):
  h1T = relu(W1eff.T @ xT [+b1 via ones-row])   [256, 2048]
  h2T = relu(W2.T   @ h1T + b2)                 [256, 2048]
  oT  =      W3.T   @ h2T + b3                  [10, 2048]
Matmuls in fp16 (full-rate PE) with fp32 PSUM accumulation.

Schedule rationale (from trace analysis):
 - The PE HAM clock gate holds the PE at 1.2GHz until ~3.4us of SUSTAINED
   busy-ness; any gap in that burst delays the 2.4GHz un-throttle.  So the
   warm-up burst runs gap-free from engine-alive (~1.7us) into the first
   data arrival, and chunk-0 is sized so its pieces arrive faster than the
   (still cold) PE consumes them.
 - HWDGE queues: first bytes land ~2.9us (runtime preamble + descgen +
   queue latency); effective 2-ring fill rate ~280GB/s.  Chunk sizes ramp
   [256,512,512,512,256]: chunk-0 needs only w1+384KB of x, so its last
   piece lands ~5.3us; steady-state chunks are PE-bound (x supply 2.7us
   per 512-chunk vs 4.3us compute).
 - Chunk-0 ships as SELF-CONTAINED per-k pieces [w1_k | x0_k] alternating
   across the two rings in consumption order; each piece's semaphore
   releases both m-matmuls of its k-tile at once.
 - The small last chunk shortens the serial relu->L2->relu->L3->store
   dependency tail; its L1 runs m-outer so the m0 relu overlaps m1.
 - Output is stored as fp16 (cast up on host; ~1e-4 extra rel err);
   stores ride SWDGE (gpsimd) so they never touch the load rings, except
   the last store which rides the by-then-idle sync ring.
"""

import numpy as np

import concourse.bacc as bacc
import concourse.bass as bass
import concourse.mybir as mybir
import concourse.tile as tile
from concourse.bass_utils import run_bass_kernel_spmd

N_CORES = 8
B = 16384
B_LOC = B // N_CORES  # 2048 batch rows per core
CS = [256, 512, 512, 512, 256]  # ramped chunk sizes
CO = [0, 256, 768, 1280, 1792]  # chunk offsets
NCHUNKS = len(CS)
KIN = 784  # folded input features (28*28)
NK = 6  # full 128-row k-tiles; tile 6 is the 16-row tail (+1 ones row)
KTAIL = KIN - NK * 128  # 16
H = 256
NOUT = 10
NWARM = 17  # warm-up matmuls bridging engine-alive -> first data, gap-free

C0W = 256 + CS[0]  # 512 cols per chunk-0 [w_k | x0_k] piece
WA_W3 = 2 * H  # col offset of w3 block in wa
WA_B2 = WA_W3 + 2 * NOUT  # col offset of b2 (2 cols)
WA_B3 = WA_B2 + 2  # col offset of b3 (1 col)
WA_COLS = WA_B3 + 1

F32 = mybir.dt.float32
F16 = mybir.dt.float16
AF = mybir.ActivationFunctionType
ALU = mybir.AluOpType


def build_nc() -> bass.Bass:
    nc = bacc.Bacc(
        "TRN2", target_bir_lowering=False, debug=False, num_devices=N_CORES
    )
    # Host-packed inputs (exact SBUF destination layouts):
    #   c0p[p][k*512+c]: c<256 -> W1eff[k*128+p, c]; c>=256 -> xT[k*128+p, c-256]
    #   t6e[p][c]: p<16: c<256 -> W1eff[768+p, c]; c>=256 -> xT[768+p, c-256]
    #              p==16: c<256 -> b1[c]; c>=256 -> 1.0
    #   wa[p][c]: c<512 k-major W2; then k-major W3 (20); then b2 (2), b3 (1)
    #   xc[i][h][p][k*512+n] -> xT[(3h+k)*128+p, CO[i+1]+n]   (chunks 1-3)
    #   x4[h][p][k*256+n]   -> xT[(3h+k)*128+p, 1792+n]       (chunk 4)
    c0p = nc.dram_tensor("c0p", [128, NK * C0W], F16, kind="ExternalInput")
    t6e = nc.dram_tensor(
        "t6e", [KTAIL + 1, 256 + B_LOC], F16, kind="ExternalInput"
    )
    wa = nc.dram_tensor("wa", [128, WA_COLS], F16, kind="ExternalInput")
    xc = nc.dram_tensor("xc", [3, 2, 128, 3 * 512], F16, kind="ExternalInput")
    x4 = nc.dram_tensor("x4", [2, 128, 3 * 256], F16, kind="ExternalInput")
    outT = nc.dram_tensor("outT", [NOUT, B_LOC], F16, kind="ExternalOutput")

    with tile.TileContext(nc) as tc:
        with (
            tc.tile_pool(name="wgt", bufs=1) as wp,
            tc.tile_pool(name="xin", bufs=3) as xp,
            tc.tile_pool(name="act", bufs=3) as hp,
            tc.tile_pool(name="osb", bufs=2) as op,
            tc.tile_pool(name="ps1", bufs=2, space="PSUM") as pp1,
            tc.tile_pool(name="ps2", bufs=2, space="PSUM") as pp2,
        ):
            # PE warm-up: small matmuls on a zeroed scratch tile, no DMA deps.
            warm = wp.tile([128, 128], F16, name="warm")
            nc.vector.memset(warm[:], 0.0)
            psw = pp1.tile([128, 512], F32, name="psw", tag="ps1_0")
            for _ in range(NWARM):
                nc.tensor.matmul(
                    psw[:, 0:128], warm[:], warm[:], start=True, stop=True
                )

            # ---- chunk-0 self-contained [w|x] pieces, alternating across the
            # two HWDGE rings in consumption order ----
            c0t = [wp.tile([128, C0W], F16, name=f"c0k{k}") for k in range(NK)]
            t6t = wp.tile([KTAIL + 1, 256 + B_LOC], F16, name="t6t")
            wat = wp.tile([128, WA_COLS], F16, name="wat")

            for k in range(NK):
                eng = nc.sync if k % 2 == 0 else nc.scalar
                eng.dma_start(out=c0t[k][:], in_=c0p[:, k * C0W : (k + 1) * C0W])
            nc.sync.dma_start(out=t6t[:], in_=t6e[:, :])
            nc.scalar.dma_start(out=wat[:], in_=wa[:, :])

            # later-chunk x prefetches (bufs=3 -> no sequencer blocking)
            xa_t = {}
            xb_t = {}
            for ci in range(1, 4):
                xa_t[ci] = xp.tile([128, 3 * 512], F16, name=f"xa{ci}", tag="xa")
                nc.sync.dma_start(out=xa_t[ci][:], in_=xc[ci - 1, 0])
                xb_t[ci] = xp.tile([128, 3 * 512], F16, name=f"xb{ci}", tag="xb")
                nc.scalar.dma_start(out=xb_t[ci][:], in_=xc[ci - 1, 1])
            xa_t[4] = xp.tile([128, 3 * 256], F16, name="xa4", tag="xa")
            nc.sync.dma_start(out=xa_t[4][:], in_=x4[0])
            xb_t[4] = xp.tile([128, 3 * 256], F16, name="xb4", tag="xb")
            nc.scalar.dma_start(out=xb_t[4][:], in_=x4[1])

            def w1_piece(k, m):
                if k == NK:
                    return t6t[0 : KTAIL + 1, m * 128 : (m + 1) * 128]
                return c0t[k][:, m * 128 : (m + 1) * 128]

            def x_piece(ci, k):
                cs = CS[ci]
                if k == NK:
                    return t6t[0 : KTAIL + 1, 256 + CO[ci] : 256 + CO[ci] + cs]
                if ci == 0:
                    return c0t[k][:, 256 : 256 + cs]
                if k < 3:
                    return xa_t[ci][:, k * cs : (k + 1) * cs]
                return xb_t[ci][:, (k - 3) * cs : (k - 2) * cs]

            # fp32 staging of b2/b3 (tensor_scalar needs fp32 scalar APs);
            # one DVE copy, far off the critical path.
            bf = wp.tile([128, 3], F32, name="bf")
            nc.vector.tensor_copy(bf[:], wat[:, WA_B2 : WA_B3 + 1])
            b2m = [bf[:, 0:1], bf[:, 1:2]]
            b3v = bf[0:NOUT, 2:3]

            # ---- batch-chunk pipeline ----
            for ci in range(NCHUNKS):
                cs = CS[ci]
                n0 = CO[ci]
                last = ci == NCHUNKS - 1

                # layer 1.  k-outer/m-inner so each arriving piece feeds both
                # m matmuls at once; the LAST chunk runs m-outer so ps1_0
                # completes early and its relu overlaps the m1 pass.
                ps1f = [
                    pp1.tile([128, 512], F32, name="ps1", tag=f"ps1_{m}")
                    for m in range(2)
                ]
                ps1 = [p[:, 0:cs] for p in ps1f]
                if not last:
                    for k in range(NK + 1):
                        xv = x_piece(ci, k)
                        for m in range(2):
                            nc.tensor.matmul(
                                ps1[m],
                                w1_piece(k, m),
                                xv,
                                start=(k == 0),
                                stop=(k == NK),
                            )
                else:
                    for m in range(2):
                        for k in range(NK + 1):
                            nc.tensor.matmul(
                                ps1[m],
                                w1_piece(k, m),
                                x_piece(ci, k),
                                start=(k == 0),
                                stop=(k == NK),
                            )

                h1 = []
                for m in range(2):
                    hf = hp.tile([128, 512], F16, name="h1", tag=f"h1_{m}")
                    h = hf[:, 0:cs]
                    if m == 0:
                        nc.scalar.activation(h, ps1[m], AF.Relu)
                    else:
                        nc.vector.tensor_scalar(h, ps1[m], 0.0, None, ALU.max)
                    h1.append(h)

                # layer 2: h2T = relu(W2.T @ h1T + b2)
                h2 = []
                for m in range(2):
                    psf = pp2.tile([128, 512], F32, name="ps2", tag=f"ps2_{m}")
                    ps = psf[:, 0:cs]
                    for k in range(2):
                        nc.tensor.matmul(
                            ps,
                            wat[:, k * H + m * 128 : k * H + (m + 1) * 128],
                            h1[k],
                            start=(k == 0),
                            stop=(k == 1),
                        )
                    hf = hp.tile([128, 512], F16, name="h2", tag=f"h2_{m}")
                    h = hf[:, 0:cs]
                    if m == 0:
                        nc.scalar.activation(h, ps, AF.Relu, bias=b2m[m])
                    else:
                        nc.vector.tensor_scalar(
                            h, ps, b2m[m], 0.0, ALU.add, ALU.max
                        )
                    h2.append(h)

                # layer 3: oT = W3.T @ h2T + b3 (shares ps2_1 bank slots)
                ps3f = pp2.tile([128, 512], F32, name="ps3", tag="ps2_1")
                ps = ps3f[0:NOUT, 0:cs]
                for k in range(2):
                    nc.tensor.matmul(
                        ps,
                        wat[:, WA_W3 + k * NOUT : WA_W3 + (k + 1) * NOUT],
                        h2[k],
                        start=(k == 0),
                        stop=(k == 1),
                    )
                obf = op.tile([NOUT, 512], F16, name="ob", tag="ob")
                ob = obf[:, 0:cs]
                nc.vector.tensor_scalar(ob, ps, b3v, None, ALU.add)
                if not last:
                    nc.gpsimd.dma_start(out=outT[:, n0 : n0 + cs], in_=ob)
                else:
                    nc.sync.dma_start(out=outT[:, n0 : n0 + cs], in_=ob)

    nc.compile()
    return nc


def _fold_conv_into_w1(conv_w: np.ndarray, W1: np.ndarray) -> np.ndarray:
    """W1eff[784, 256] such that x @ W1eff == conv_flat(x, conv_w) @ W1."""
    W1v = W1.astype(np.float64).reshape(26, 26, W1.shape[1])
    cw = conv_w.astype(np.float64)
    acc = np.zeros((28, 28, W1.shape[1]), np.float64)
    for di in range(3):
        for dj in range(3):
            acc[di : di + 26, dj : dj + 26, :] += cw[di, dj] * W1v
    return acc.reshape(KIN, W1.shape[1]).astype(np.float32)


def _pack_kmajor(w: np.ndarray, kpad: int) -> np.ndarray:
    """[K, C] -> [128, (K/128)*C] with row-block k at column block k."""
    k, c = w.shape
    wp = np.zeros((kpad, c), w.dtype)
    wp[:k] = w
    return np.ascontiguousarray(
        wp.reshape(kpad // 128, 128, c).transpose(1, 0, 2).reshape(128, -1)
    )


def _run(inputs: dict, trace: bool = False, tmpdir: str | None = None):
    x = np.asarray(inputs["x"], dtype=np.float32)
    w1e = _fold_conv_into_w1(
        np.asarray(inputs["conv_w"]), np.asarray(inputs["W1"])
    ).astype(np.float16)
    w2P = _pack_kmajor(np.asarray(inputs["W2"], np.float16), H)
    w3P = _pack_kmajor(np.asarray(inputs["W3"], np.float16), H)
    wa = np.zeros((128, WA_COLS), np.float16)
    wa[:, : 2 * H] = w2P
    wa[:, WA_W3 : WA_W3 + 2 * NOUT] = w3P
    wa[:, WA_B2 : WA_B2 + 2] = (
        np.asarray(inputs["b2"], np.float16).reshape(2, 128).T
    )
    wa[:NOUT, WA_B3] = np.asarray(inputs["b3"], np.float16)
    b1 = np.asarray(inputs["b1"], np.float16)

    nc = build_nc()
    in_maps = []
    for c in range(N_CORES):
        xs = x[c * B_LOC : (c + 1) * B_LOC].astype(np.float16)  # [2048, 784]
        xsT = np.ascontiguousarray(xs.T)  # [784, 2048]
        c0pc = np.empty((128, NK * C0W), np.float16)
        for k in range(NK):
            c0pc[:, k * C0W : k * C0W + 256] = w1e[k * 128 : (k + 1) * 128]
            c0pc[:, k * C0W + 256 : (k + 1) * C0W] = xsT[
                k * 128 : (k + 1) * 128, : CS[0]
            ]
        t6c = np.empty((KTAIL + 1, 256 + B_LOC), np.float16)
        t6c[:KTAIL, :256] = w1e[NK * 128 :]
        t6c[KTAIL, :256] = b1
        t6c[:KTAIL, 256:] = xsT[NK * 128 :, :]
        t6c[KTAIL, 256:] = 1.0
        xcc = np.empty((3, 2, 128, 3 * 512), np.float16)
        for ci in range(1, 4):
            for h in range(2):
                blk = xsT[
                    3 * h * 128 : 3 * (h + 1) * 128, CO[ci] : CO[ci] + 512
                ]  # [384, 512]
                xcc[ci - 1, h] = (
                    blk.reshape(3, 128, 512).transpose(1, 0, 2).reshape(128, -1)
                )
        x4c = np.empty((2, 128, 3 * 256), np.float16)
        for h in range(2):
            blk = xsT[3 * h * 128 : 3 * (h + 1) * 128, CO[4] : CO[4] + 256]
            x4c[h] = blk.reshape(3, 128, 256).transpose(1, 0, 2).reshape(128, -1)
        in_maps.append(
            {"c0p": c0pc, "t6e": t6c, "wa": wa, "xc": xcc, "x4": x4c}
        )

    try:
        res = run_bass_kernel_spmd(
            nc, in_maps, list(range(N_CORES)), trace=trace, tmpdir=tmpdir
        )
    except Exception:
        # A prior session can leave a NeuronCore wedged
        # (NRT_EXEC_UNIT_UNRECOVERABLE); a retry with core reset recovers.
        import os

        os.environ.setdefault("NEURON_RT_RESET_CORES", "1")
        res = run_bass_kernel_spmd(
            nc, in_maps, list(range(N_CORES)), trace=trace, tmpdir=tmpdir
        )
    out = np.concatenate(
        [r["outT"].astype(np.float32).T for r in res.results], axis=0
    )
    return np.ascontiguousarray(out), res


def kernel(**inputs) -> np.ndarray:
    out, _ = _run(inputs, trace=False)
    return out


# revision 11
# speedup vs baseline: 1.0384x; 1.0384x over previous
"""Trainium2 Bass kernel for nn_DigitConvolutionalModel (3x3 conv + 3-layer MLP).

Math: out = relu(relu(conv3x3(x) @ W1 + b1) @ W2 + b2) @ W3 + b3.

The 3x3 valid conv is linear, so on host we fold it into the first FC:
  h1 = relu(x @ W1eff + b1)  with  W1eff = A @ W1 : [784, 256].
K = 784 is used EXACTLY (6 full 128-row k-tiles + one 16-row tail tile).
b1 rides the tail matmul as a 17th row (ones row in x-tail, b1 row in
W-tail), so L1 needs no separate bias add.  b2/b3 ride as fp16 columns of
the wa tensor (one DVE copy stages them to fp32).

Sharding: pure data parallel over the batch across 8 cores (2048 rows each).
Feature-major 3-layer MLP (activations transposed; zero on-device transposes):
  h1T = relu(W1eff.T @ xT [+b1 via ones-row])   [256, 2048]
  h2T = relu(W2.T   @ h1T + b2)                 [256, 2048]
  oT  =      W3.T   @ h2T + b3                  [10, 2048]
Matmuls in fp16 (full-rate PE) with fp32 PSUM accumulation.

Schedule rationale (from trace analysis):
 - The PE HAM clock gate holds the PE at 1.2GHz until ~3.4us of SUSTAINED
   busy-ness; any gap in that burst delays the 2.4GHz un-throttle.  So the
   warm-up burst runs gap-free from engine-alive (~1.7us) into the first
   data arrival, and chunk-0 is sized so its pieces arrive faster than the
   (still cold) PE consumes them.
 - HWDGE queues: first bytes land ~2.9us (runtime preamble + descgen +
   queue latency); effective 2-ring fill rate ~280GB/s.  Chunk sizes ramp
   [256,512,512,512,256]: chunk-0 needs only w1+384KB of x, so its last
   piece lands ~5.3us; steady-state chunks are PE-bound (x supply 2.7us
   per 512-chunk vs 4.3us compute).
 - Chunk-0 ships as SELF-CONTAINED per-k pieces [w1_k | x0_k] alternating
   across the two rings in consumption order; each piece's semaphore
   releases both m-matmuls of its k-tile at once.
 - The small last chunk shortens the serial relu->L2->relu->L3->store
   dependency tail; its L1 runs m-outer so the m0 relu overlaps m1.
 - Output is stored as fp16 (cast up on host; ~1e-4 extra rel err);
   stores ride SWDGE (gpsimd) so they never touch the load rings, except
   the last store which rides the by-then-idle sync ring.
"""

import numpy as np

import concourse.bacc as bacc
import concourse.bass as bass
import concourse.mybir as mybir
import concourse.tile as tile
from concourse.bass_utils import run_bass_kernel_spmd

N_CORES = 8
B = 16384
B_LOC = B // N_CORES  # 2048 batch rows per core
CS = [256, 512, 512, 512, 256]  # ramped chunk sizes
CO = [0, 256, 768, 1280, 1792]  # chunk offsets
NCHUNKS = len(CS)
KIN = 784  # folded input features (28*28)
NK = 6  # full 128-row k-tiles; tile 6 is the 16-row tail (+1 ones row)
KTAIL = KIN - NK * 128  # 16
H = 256
NOUT = 10
NWARM = 24  # warm-up matmuls bridging engine-alive -> first data, gap-free

C0W = 256 + CS[0]  # 512 cols per chunk-0 [w_k | x0_k] piece
WA_W3 = 2 * H  # col offset of w3 block in wa
WA_B2 = WA_W3 + 2 * NOUT  # col offset of b2 (2 cols)
WA_B3 = WA_B2 + 2  # col offset of b3 (1 col)
WA_COLS = WA_B3 + 1

F32 = mybir.dt.float32
F16 = mybir.dt.float16
AF = mybir.ActivationFunctionType
ALU = mybir.AluOpType


def build_nc() -> bass.Bass:
    nc = bacc.Bacc(
        "TRN2", target_bir_lowering=False, debug=False, num_devices=N_CORES
    )
    # Host-packed inputs (exact SBUF destination layouts):
    #   c0p[p][k*512+c]: c<256 -> W1eff[k*128+p, c]; c>=256 -> xT[k*128+p, c-256]
    #   t6e[p][c]: p<16: c<256 -> W1eff[768+p, c]; c>=256 -> xT[768+p, c-256]
    #              p==16: c<256 -> b1[c]; c>=256 -> 1.0
    #   wa[p][c]: c<512 k-major W2; then k-major W3 (20); then b2 (2), b3 (1)
    #   xc[i][h][p][k*512+n] -> xT[(3h+k)*128+p, CO[i+1]+n]   (chunks 1-3)
    #   x4[h][p][k*256+n]   -> xT[(3h+k)*128+p, 1792+n]       (chunk 4)
    c0p = nc.dram_tensor("c0p", [128, NK * C0W], F16, kind="ExternalInput")
    t6e = nc.dram_tensor(
        "t6e", [KTAIL + 1, 256 + B_LOC], F16, kind="ExternalInput"
    )
    wa = nc.dram_tensor("wa", [128, WA_COLS], F16, kind="ExternalInput")
    xc = nc.dram_tensor("xc", [3, 2, 128, 3 * 512], F16, kind="ExternalInput")
    x4 = nc.dram_tensor("x4", [2, 128, 3 * 256], F16, kind="ExternalInput")
    outT = nc.dram_tensor("outT", [NOUT, B_LOC], F16, kind="ExternalOutput")

    with tile.TileContext(nc) as tc:
        with (
            tc.tile_pool(name="wgt", bufs=1) as wp,
            tc.tile_pool(name="xin", bufs=3) as xp,
            tc.tile_pool(name="act", bufs=3) as hp,
            tc.tile_pool(name="osb", bufs=2) as op,
            tc.tile_pool(name="ps1", bufs=2, space="PSUM") as pp1,
            tc.tile_pool(name="ps2", bufs=2, space="PSUM") as pp2,
        ):
            # PE warm-up: small matmuls on a zeroed scratch tile, no DMA deps.
            warm = wp.tile([128, 128], F16, name="warm")
            nc.vector.memset(warm[:], 0.0)
            psw = pp1.tile([128, 512], F32, name="psw", tag="ps1_0")
            for _ in range(NWARM):
                nc.tensor.matmul(
                    psw[:, 0:128], warm[:], warm[:], start=True, stop=True
                )

            # ---- chunk-0 self-contained [w|x] pieces, alternating across the
            # two HWDGE rings in consumption order ----
            c0t = [wp.tile([128, C0W], F16, name=f"c0k{k}") for k in range(NK)]
            t6t = wp.tile([KTAIL + 1, 256 + B_LOC], F16, name="t6t")
            wat = wp.tile([128, WA_COLS], F16, name="wat")

            # t6e has only 17 partition rows -> its descriptors land on very
            # few SDMA engines; it MUST go first, while the rings are empty.
            # Each ring sustains only ~3 in-flight transfers, so transfer
            # order is also issue order.
            nc.scalar.dma_start(out=t6t[:], in_=t6e[:, :])
            for k in range(NK):
                eng = nc.sync if k % 2 == 0 else nc.scalar
                eng.dma_start(out=c0t[k][:], in_=c0p[:, k * C0W : (k + 1) * C0W])
            nc.sync.dma_start(out=wat[:], in_=wa[:, :])

            # later-chunk x prefetches (3 tagged bufs = exactly 3 users per
            # tag, x4 gets dedicated tiles -> no dma_start ever blocks its
            # sequencer on buffer reuse)
            xa_t = {}
            xb_t = {}
            for ci in range(1, 4):
                xa_t[ci] = xp.tile([128, 3 * 512], F16, name=f"xa{ci}", tag="xa")
                nc.sync.dma_start(out=xa_t[ci][:], in_=xc[ci - 1, 0])
                xb_t[ci] = xp.tile([128, 3 * 512], F16, name=f"xb{ci}", tag="xb")
                nc.scalar.dma_start(out=xb_t[ci][:], in_=xc[ci - 1, 1])
            xa_t[4] = xp.tile([128, 3 * 256], F16, name="xa4")
            nc.sync.dma_start(out=xa_t[4][:], in_=x4[0])
            xb_t[4] = xp.tile([128, 3 * 256], F16, name="xb4")
            nc.scalar.dma_start(out=xb_t[4][:], in_=x4[1])

            def w1_piece(k, m):
                if k == NK:
                    return t6t[0 : KTAIL + 1, m * 128 : (m + 1) * 128]
                return c0t[k][:, m * 128 : (m + 1) * 128]

            def x_piece(ci, k):
                cs = CS[ci]
                if k == NK:
                    return t6t[0 : KTAIL + 1, 256 + CO[ci] : 256 + CO[ci] + cs]
                if ci == 0:
                    return c0t[k][:, 256 : 256 + cs]
                if k < 3:
                    return xa_t[ci][:, k * cs : (k + 1) * cs]
                return xb_t[ci][:, (k - 3) * cs : (k - 2) * cs]

            # fp32 staging of b2/b3 (tensor_scalar needs fp32 scalar APs);
            # one DVE copy, far off the critical path.
            bf = wp.tile([128, 3], F32, name="bf")
            nc.vector.tensor_copy(bf[:], wat[:, WA_B2 : WA_B3 + 1])
            b2m = [bf[:, 0:1], bf[:, 1:2]]
            b3v = bf[0:NOUT, 2:3]

            # ---- batch-chunk pipeline ----
            for ci in range(NCHUNKS):
                cs = CS[ci]
                n0 = CO[ci]
                last = ci == NCHUNKS - 1

                # layer 1.  k-outer/m-inner so each arriving piece feeds both
                # m matmuls at once; the LAST chunk runs m-outer so ps1_0
                # completes early and its relu overlaps the m1 pass.
                ps1f = [
                    pp1.tile([128, 512], F32, name="ps1", tag=f"ps1_{m}")
                    for m in range(2)
                ]
                ps1 = [p[:, 0:cs] for p in ps1f]
                if not last:
                    for k in range(NK + 1):
                        xv = x_piece(ci, k)
                        for m in range(2):
                            nc.tensor.matmul(
                                ps1[m],
                                w1_piece(k, m),
                                xv,
                                start=(k == 0),
                                stop=(k == NK),
                            )
                else:
                    for m in range(2):
                        for k in range(NK + 1):
                            nc.tensor.matmul(
                                ps1[m],
                                w1_piece(k, m),
                                x_piece(ci, k),
                                start=(k == 0),
                                stop=(k == NK),
                            )

                h1 = []
                for m in range(2):
                    hf = hp.tile([128, 512], F16, name="h1", tag=f"h1_{m}")
                    h = hf[:, 0:cs]
                    if m == 0:
                        nc.scalar.activation(h, ps1[m], AF.Relu)
                    else:
                        nc.vector.tensor_scalar(h, ps1[m], 0.0, None, ALU.max)
                    h1.append(h)

                # layer 2: h2T = relu(W2.T @ h1T + b2)
                h2 = []
                for m in range(2):
                    psf = pp2.tile([128, 512], F32, name="ps2", tag=f"ps2_{m}")
                    ps = psf[:, 0:cs]
                    for k in range(2):
                        nc.tensor.matmul(
                            ps,
                            wat[:, k * H + m * 128 : k * H + (m + 1) * 128],
                            h1[k],
                            start=(k == 0),
                            stop=(k == 1),
                        )
                    hf = hp.tile([128, 512], F16, name="h2", tag=f"h2_{m}")
                    h = hf[:, 0:cs]
                    if m == 0:
                        nc.scalar.activation(h, ps, AF.Relu, bias=b2m[m])
                    else:
                        nc.vector.tensor_scalar(
                            h, ps, b2m[m], 0.0, ALU.add, ALU.max
                        )
                    h2.append(h)

                # layer 3: oT = W3.T @ h2T + b3 (shares ps2_1 bank slots)
                ps3f = pp2.tile([128, 512], F32, name="ps3", tag="ps2_1")
                ps = ps3f[0:NOUT, 0:cs]
                for k in range(2):
                    nc.tensor.matmul(
                        ps,
                        wat[:, WA_W3 + k * NOUT : WA_W3 + (k + 1) * NOUT],
                        h2[k],
                        start=(k == 0),
                        stop=(k == 1),
                    )
                obf = op.tile([NOUT, 512], F16, name="ob", tag="ob")
                ob = obf[:, 0:cs]
                nc.vector.tensor_scalar(ob, ps, b3v, None, ALU.add)
                if not last:
                    nc.gpsimd.dma_start(out=outT[:, n0 : n0 + cs], in_=ob)
                else:
                    nc.sync.dma_start(out=outT[:, n0 : n0 + cs], in_=ob)

    nc.compile()
    return nc


def _fold_conv_into_w1(conv_w: np.ndarray, W1: np.ndarray) -> np.ndarray:
    """W1eff[784, 256] such that x @ W1eff == conv_flat(x, conv_w) @ W1."""
    W1v = W1.astype(np.float64).reshape(26, 26, W1.shape[1])
    cw = conv_w.astype(np.float64)
    acc = np.zeros((28, 28, W1.shape[1]), np.float64)
    for di in range(3):
        for dj in range(3):
            acc[di : di + 26, dj : dj + 26, :] += cw[di, dj] * W1v
    return acc.reshape(KIN, W1.shape[1]).astype(np.float32)


def _pack_kmajor(w: np.ndarray, kpad: int) -> np.ndarray:
    """[K, C] -> [128, (K/128)*C] with row-block k at column block k."""
    k, c = w.shape
    wp = np.zeros((kpad, c), w.dtype)
    wp[:k] = w
    return np.ascontiguousarray(
        wp.reshape(kpad // 128, 128, c).transpose(1, 0, 2).reshape(128, -1)
    )


def _run(inputs: dict, trace: bool = False, tmpdir: str | None = None):
    x = np.asarray(inputs["x"], dtype=np.float32)
    w1e = _fold_conv_into_w1(
        np.asarray(inputs["conv_w"]), np.asarray(inputs["W1"])
    ).astype(np.float16)
    w2P = _pack_kmajor(np.asarray(inputs["W2"], np.float16), H)
    w3P = _pack_kmajor(np.asarray(inputs["W3"], np.float16), H)
    wa = np.zeros((128, WA_COLS), np.float16)
    wa[:, : 2 * H] = w2P
    wa[:, WA_W3 : WA_W3 + 2 * NOUT] = w3P
    wa[:, WA_B2 : WA_B2 + 2] = (
        np.asarray(inputs["b2"], np.float16).reshape(2, 128).T
    )
    wa[:NOUT, WA_B3] = np.asarray(inputs["b3"], np.float16)
    b1 = np.asarray(inputs["b1"], np.float16)

    nc = build_nc()
    in_maps = []
    for c in range(N_CORES):
        xs = x[c * B_LOC : (c + 1) * B_LOC].astype(np.float16)  # [2048, 784]
        xsT = np.ascontiguousarray(xs.T)  # [784, 2048]
        c0pc = np.empty((128, NK * C0W), np.float16)
        for k in range(NK):
            c0pc[:, k * C0W : k * C0W + 256] = w1e[k * 128 : (k + 1) * 128]
            c0pc[:, k * C0W + 256 : (k + 1) * C0W] = xsT[
                k * 128 : (k + 1) * 128, : CS[0]
            ]
        t6c = np.empty((KTAIL + 1, 256 + B_LOC), np.float16)
        t6c[:KTAIL, :256] = w1e[NK * 128 :]
        t6c[KTAIL, :256] = b1
        t6c[:KTAIL, 256:] = xsT[NK * 128 :, :]
        t6c[KTAIL, 256:] = 1.0
        xcc = np.empty((3, 2, 128, 3 * 512), np.float16)
        for ci in range(1, 4):
            for h in range(2):
                blk = xsT[
                    3 * h * 128 : 3 * (h + 1) * 128, CO[ci] : CO[ci] + 512
                ]  # [384, 512]
                xcc[ci - 1, h] = (
                    blk.reshape(3, 128, 512).transpose(1, 0, 2).reshape(128, -1)
                )
        x4c = np.empty((2, 128, 3 * 256), np.float16)
        for h in range(2):
            blk = xsT[3 * h * 128 : 3 * (h + 1) * 128, CO[4] : CO[4] + 256]
            x4c[h] = blk.reshape(3, 128, 256).transpose(1, 0, 2).reshape(128, -1)
        in_maps.append(
            {"c0p": c0pc, "t6e": t6c, "wa": wa, "xc": xcc, "x4": x4c}
        )

    try:
        res = run_bass_kernel_spmd(
            nc, in_maps, list(range(N_CORES)), trace=trace, tmpdir=tmpdir
        )
    except Exception:
        # A prior session can leave a NeuronCore wedged
        # (NRT_EXEC_UNIT_UNRECOVERABLE); a retry with core reset recovers.
        import os

        os.environ.setdefault("NEURON_RT_RESET_CORES", "1")
        res = run_bass_kernel_spmd(
            nc, in_maps, list(range(N_CORES)), trace=trace, tmpdir=tmpdir
        )
    out = np.concatenate(
        [r["outT"].astype(np.float32).T for r in res.results], axis=0
    )
    return np.ascontiguousarray(out), res


def kernel(**inputs) -> np.ndarray:
    out, _ = _run(inputs, trace=False)
    return out
